# revision 24
# baseline (speedup 1.0000x reference)
"""YOLOv1 loss (nn_LossModul_16277926052544) on 8 TRN2 NeuronCores.

Pure data parallel: batch 8192 -> 8 shards of 1024. Each core computes a
partial loss over its shard; host sums the 8x128 partials.

v10 design (75us baseline -> 62.7 -> 58.3 -> 53.2 -> this). Findings:
  * ~350ns fixed cost per Vector op (SBUF init + semaphore instr) and a
    ~16us fixed NEFF startup+teardown dominate; one 392-cell tile (T=1)
    minimizes op count.
  * inputs host-cast bf16 (TT 2x_1p, TS 4x; STT/copy_predicated are 1x on
    cayman, so plain TT/TS preferred).  cls channels (20 of 35) are fp8
    e4m3: the cls stream halves again, and quantization only biases the
    cls term ~0.2% (tolerance 2e-2).
  * cls diff (pcls-tcls) computed BY THE DMA: host stores negated
    targets; SWDGE accum_op=add DMAs (SDMA CCE) cast fp8->bf16 and add
    onto the loaded pred rows.  CCE mangles transfers over ~2048
    elems/partition -> 5-row accum chunks.
  * DMA ordering: geometry rows stream first (Vector starts ~9.5us); the
    cls stream is gated behind them by a 1-element Vector write that
    creates a WAW edge (program order alone gets rescheduled).
  * xy rows host-prescaled by R=1/S and |.| taken in-place via a bf16
    sign-bit AND (bitcast u16): squares kill the sign, masks fold the
    7*sqrt5 constant back.
  * no GpSimd compute (each GP op ~1.5-2us fixed, and GP ops block DVE
    2-port ops via the shared SBUF port); GP only issues SWDGE DMAs.
  * ACT holds one table set (warm-up Sqrt during the DMA ramp; Square
    lives in the sqrt set) and runs Sqrt(scale=5 folds lambda) plus three
    Square+accum ops that pipeline behind Vector.
"""
import sys

for _p in ("/opt/trn_rl_repo",):
    if _p not in sys.path:
        sys.path.insert(0, _p)

import numpy as np
import ml_dtypes
from contextlib import ExitStack

import concourse.bass as bass  # noqa: F401  (registers engines)
from concourse import bacc, mybir
from concourse import bass_utils
import concourse.tile as tile

N_CORES = 8
BATCH = 8192
S = 7
P = 128
F = (BATCH // N_CORES) * S * S // P           # 392 cells per partition
R = 1.0 / S
EPS5 = 5e-6                                   # 5 * EPS (lambda folded)
SQRT5_7 = float(7.0 * np.sqrt(5.0))           # xy mask: host prescaled by R
SQH = float(np.sqrt(0.5))

CG = 15                                       # geometry rows per cell
CC = 20                                       # cls rows per cell

f32 = mybir.dt.float32
bf16 = mybir.dt.bfloat16
u16 = mybir.dt.uint16
u32 = mybir.dt.uint32
f8 = mybir.dt.float8e4
Alu = mybir.AluOpType
Act = mybir.ActivationFunctionType

_CACHE = {}


def _build_body(tc, ctx, xg, xc, tn, out_ap):
    nc = tc.nc
    wk = ctx.enter_context(tc.tile_pool(name="wk", bufs=1))
    stats = wk.tile([P, 4], f32)
    eps5c = wk.tile([P, 1], f32)                # bias const for Sqrt
    nc.gpsimd.memset(eps5c[:], EPS5)
    warm = wk.tile([P, 1], f32)                 # pulls the sqrt table set in
    nc.scalar.activation(warm[:], eps5c[:], Act.Sqrt)

    # xg rows: 0:2 R*pxy1 | 2:4 R*pxy2 | 4:7 pw1,ph1,pc1 | 7:10 pw2,ph2,pc2
    #          10:12 tw,th | 12 tc | 13:15 -R*tx,-R*ty
    xp = wk.tile([P, CG, F], bf16, tag="x")
    nc.sync.dma_start(xp[:, 4:15], xg[:, 4:15])
    nc.sync.dma_start(xp[:, 0:4], xg[:, 0:4])

    dcls = wk.tile([P, CC, F], bf16, tag="dcls")
    # Gate the cls stream on the geometry DMAs so it doesn't steal SDMA
    # ring bandwidth from them: a 1-element Vector write into dcls creates
    # the WAW edge the scheduler respects.
    nc.vector.tensor_copy(dcls[:, 0, 0:1], xp[:, 0, 0:1])
    # fp8 pcls cast to bf16 by the load; fp8 -tcls cast+added by the CCE
    # accumulates (5-row chunks stay under the ~2048 elem/partition limit)
    nc.gpsimd.dma_start(dcls[:], xc[:, 0:20])
    for k in range(4):
        nc.gpsimd.dma_start(dcls[:, 5 * k:5 * k + 5],
                            tn[:, 5 * k:5 * k + 5], accum_op=Alu.add)

    pxy = xp[:, 0:4].rearrange("p (b c) f -> p b c f", b=2)
    pbox = xp[:, 4:10].rearrange("p (b c) f -> p b c f", b=2)
    pwh = pbox[:, :, 0:2, :]                                  # [P,2,2,F]
    pc = pbox[:, :, 2, :]                                     # [P,2,F]
    twh = xp[:, 10:12]                                        # [P,2,F]
    twh_b = xp[:, 10:12].unsqueeze(1).broadcast_to([P, 2, 2, F])
    tcf = xp[:, 12]                                           # [P,F]
    ntxy_b = xp[:, 13:15].unsqueeze(1).broadcast_to([P, 2, 2, F])

    def bc(ap_pf, k):
        return ap_pf.unsqueeze(1).broadcast_to([P, k, F])

    flat = lambda a: a.rearrange("p b c f -> p (b c f)")

    # ---------------- masks (TS, 4x) ----------------
    mo = wk.tile([P, F], bf16, tag="mo")
    nc.vector.tensor_scalar(mo[:], tcf, 0.0, None, op0=Alu.is_gt)
    ms5 = wk.tile([P, F], bf16, tag="ms5")
    nc.vector.tensor_scalar(ms5[:], tcf, 0.0, SQRT5_7, op0=Alu.is_gt,
                            op1=Alu.mult)
    mnh = wk.tile([P, F], bf16, tag="mnh")
    nc.vector.tensor_scalar(mnh[:], tcf, 0.0, SQH, op0=Alu.is_le,
                            op1=Alu.mult)

    # ---------------- geometry ----------------
    # dxy = |R*(pxy - txy)|: xy rows are host-prescaled by R, and the xy
    # loss squares the selected value, so the abs can run in place
    dxy = wk.tile([P, 2, 2, F], bf16, tag="dxy")
    nc.vector.tensor_tensor(dxy[:], pxy, ntxy_b, op=Alu.add)
    nc.vector.tensor_scalar(
        flat(dxy[:]).bitcast(u16), flat(dxy[:]).bitcast(u16), 0x7FFF,
        None, op0=Alu.bitwise_and)
    sth = wk.tile([P, 2, F], bf16, tag="sth")          # twh/2
    nc.vector.tensor_scalar(sth[:], twh, 0.5, None, op0=Alu.mult)
    hp = wk.tile([P, 2, 2, F], bf16, tag="hp")         # pwh/2
    nc.vector.tensor_scalar(hp[:], pwh, 0.5, None, op0=Alu.mult)
    s = wk.tile([P, 2, 2, F], bf16, tag="s")           # (pwh+twh)/2
    nc.vector.tensor_tensor(
        s[:], hp[:], sth[:].unsqueeze(1).broadcast_to([P, 2, 2, F]),
        op=Alu.add)
    m = wk.tile([P, 2, 2, F], bf16, tag="m")           # s - R|dxy|
    nc.vector.tensor_sub(flat(m[:]), flat(s[:]), flat(dxy[:]))
    mwh = wk.tile([P, 2, 2, F], bf16, tag="mwh")       # min(pwh, twh)
    nc.vector.tensor_tensor(mwh[:], pwh, twh_b, op=Alu.min)
    ln = wk.tile([P, 2, 2, F], bf16, tag="ln")         # clamped overlap
    nc.vector.scalar_tensor_tensor(flat(ln[:]), flat(m[:]), 0.0,
                                   flat(mwh[:]), op0=Alu.max, op1=Alu.min)

    ID = wk.tile([P, 4, F], bf16, tag="ID")            # I1 I2 D1 D2
    nc.vector.tensor_mul(ID[:, 0:2], ln[:, :, 0, :], ln[:, :, 1, :])
    pA = wk.tile([P, 2, F], bf16, tag="pA")
    nc.vector.tensor_mul(pA[:], pbox[:, :, 0, :], pbox[:, :, 1, :])
    tA = wk.tile([P, 1, F], bf16, tag="tA")
    nc.vector.tensor_mul(tA[:], xp[:, 10:11], xp[:, 11:12])
    PT = wk.tile([P, 2, F], bf16, tag="PT")
    nc.vector.tensor_tensor(PT[:], pA[:], tA[:].broadcast_to([P, 2, F]),
                            op=Alu.add)
    nc.vector.tensor_sub(ID[:, 2:4], PT[:], ID[:, 0:2])

    g = wk.tile([P, 2, F], bf16, tag="g")
    nc.vector.tensor_mul(g[:, 0], ID[:, 0], ID[:, 3])
    nc.vector.tensor_mul(g[:, 1], ID[:, 1], ID[:, 2])
    resp = wk.tile([P, F], u32, tag="resp")            # 1 -> box1
    nc.vector.tensor_tensor(resp[:], g[:, 0], g[:, 1], op=Alu.is_gt)

    # ---------------- select responsible box ----------------
    # the wh select goes first: it feeds the ACT Sqrt round-trip, the
    # longest cross-engine chain hanging off resp
    sel = wk.tile([P, 7, F], bf16, tag="sel")
    nc.vector.tensor_copy(sel[:, 2:5], xp[:, 7:10])    # box2 defaults
    nc.vector.copy_predicated(sel[:, 2:5], bc(resp[:], 3), xp[:, 4:7])
    # work tile rows: 0:20 mo*dcls | 20:22 7sqrt5*mo*dxy_sel | 22:24 dwh
    #                 24 mo*(c_sel-iou) | 25:27 sqrt(.5)*(1-mo)*pc
    W = wk.tile([P, 27, F], bf16, tag="W")
    sq = wk.tile([P, 4, F], bf16, tag="sq")            # mo*selwh | mo*twh
    nc.vector.tensor_mul(sq[:, 0:2], sel[:, 2:4], bc(mo[:], 2))
    nc.vector.tensor_mul(sq[:, 2:4], twh, bc(mo[:], 2))
    # sqrt(5*mo*wh + 5eps): lambda_coord folds into the free affine
    nc.scalar.activation(sq[:], sq[:], Act.Sqrt, bias=eps5c[:], scale=5.0)

    nc.vector.tensor_copy(sel[:, 0:2], dxy[:, 1])
    nc.vector.tensor_copy(sel[:, 5:7], ID[:, 1:4:2])
    nc.vector.copy_predicated(sel[:, 0:2], bc(resp[:], 2), dxy[:, 0])
    nc.vector.copy_predicated(sel[:, 5:7], bc(resp[:], 2), ID[:, 0:3:2])

    # first cls half: the DMA-computed diffs streamed in during geometry,
    # and ACT can start its big Square while Vector runs the conf chain
    nc.vector.tensor_mul(W[:, 0:10], dcls[:, 0:10], bc(mo[:], 10))
    nc.scalar.activation(W[:, 0:10], W[:, 0:10], Act.Square,
                         accum_out=stats[:, 1:2])

    nc.vector.tensor_mul(W[:, 20:22], sel[:, 0:2], bc(ms5[:], 2))
    nc.vector.tensor_sub(W[:, 22:24], sq[:, 0:2], sq[:, 2:4])
    Dsel = wk.tile([P, F], f32, tag="Dsel")
    nc.vector.tensor_copy(Dsel[:], sel[:, 6])
    rcp = wk.tile([P, F], f32, tag="rcp")
    nc.vector.reciprocal_approx_fast(rcp[:], Dsel[:])
    iou = wk.tile([P, F], bf16, tag="iou")
    nc.vector.tensor_mul(iou[:], sel[:, 5], rcp[:])
    cd = wk.tile([P, F], bf16, tag="cd")               # c_sel - iou
    nc.vector.tensor_sub(cd[:], sel[:, 4], iou[:])
    nc.vector.tensor_mul(W[:, 24], cd[:], mo[:])
    nc.vector.tensor_mul(W[:, 25:27], pc, bc(mnh[:], 2))
    nc.scalar.activation(W[:, 20:27], W[:, 20:27], Act.Square,
                         accum_out=stats[:, 0:1])

    # second cls half split in two so the last ACT Square after Vector's
    # final mul is half as long
    nc.vector.tensor_mul(W[:, 10:15], dcls[:, 10:15], bc(mo[:], 5))
    nc.scalar.activation(W[:, 10:15], W[:, 10:15], Act.Square,
                         accum_out=stats[:, 2:3])
    nc.vector.tensor_mul(W[:, 15:20], dcls[:, 15:20], bc(mo[:], 5))
    nc.scalar.activation(W[:, 15:20], W[:, 15:20], Act.Square,
                         accum_out=stats[:, 3:4])

    total = wk.tile([P, 1], f32)
    nc.vector.tensor_scalar(warm[:], warm[:], 0.0, None,
                            op0=Alu.mult)              # keep warm read live
    nc.vector.tensor_reduce(total[:], stats[:],
                            axis=mybir.AxisListType.X, op=Alu.add)
    nc.sync.dma_start(out_ap, total[:])


def _build():
    if "nc" in _CACHE:
        return _CACHE["nc"]
    nc = bacc.Bacc("TRN2", target_bir_lowering=False, debug=False)
    xg = nc.dram_tensor("xg", [P, CG, F], bf16, kind="ExternalInput")
    xc = nc.dram_tensor("xc", [P, CC, F], f8, kind="ExternalInput")
    tn = nc.dram_tensor("tn", [P, CC, F], f8, kind="ExternalInput")
    out = nc.dram_tensor("out", [P, 1], f32, kind="ExternalOutput")
    with tile.TileContext(nc) as tc, ExitStack() as ctx:
        _build_body(tc, ctx, xg.ap(), xc.ap(), tn.ap(), out.ap())
    nc.compile()
    _CACHE["nc"] = nc
    return nc


def _shard(predicts, targets):
    """Full f32 inputs -> per-core (xg bf16, xc fp8, tn fp8) arrays."""
    bpc = BATCH // N_CORES
    xgs, xcs, tns = [], [], []
    for i in range(N_CORES):
        p = np.asarray(predicts[i * bpc:(i + 1) * bpc], dtype=np.float32)
        g = np.asarray(targets[i * bpc:(i + 1) * bpc], dtype=np.float32)
        pm = np.moveaxis(p.reshape(P, F, 30), 2, 1)   # [P,30,F]
        gm = np.moveaxis(g.reshape(P, F, 30), 2, 1)
        xg = np.empty((P, CG, F), dtype=np.float32)
        xg[:, 0:2] = R * pm[:, 0:2]     # R*pxy1
        xg[:, 2:4] = R * pm[:, 5:7]     # R*pxy2
        xg[:, 4:7] = pm[:, 2:5]         # pw1 ph1 pc1
        xg[:, 7:10] = pm[:, 7:10]       # pw2 ph2 pc2
        xg[:, 10:12] = gm[:, 2:4]       # tw th
        xg[:, 12] = gm[:, 4]            # tconf
        xg[:, 13:15] = -R * gm[:, 0:2]  # -R*tx -R*ty
        xgs.append(xg.astype(ml_dtypes.bfloat16))
        xcs.append(np.ascontiguousarray(pm[:, 10:30])
                   .astype(ml_dtypes.float8_e4m3))
        tns.append(np.ascontiguousarray(-gm[:, 10:30])
                   .astype(ml_dtypes.float8_e4m3))
    return xgs, xcs, tns


def run(predicts, targets, trace=False, **trace_kwargs):
    nc = _build()
    xgs, xcs, tns = _shard(predicts, targets)
    in_maps = [{"xg": xgs[i], "xc": xcs[i], "tn": tns[i]}
               for i in range(N_CORES)]
    res = bass_utils.run_bass_kernel_spmd(
        nc, in_maps, core_ids=list(range(N_CORES)), trace=trace,
        **trace_kwargs)
    partial = np.zeros((), dtype=np.float64)
    for r in res.results:
        partial += np.asarray(r["out"], dtype=np.float64).sum()
    return np.float32(partial), res


def kernel(predicts, targets):
    out, _ = run(predicts, targets, trace=False)
    return out


# revision 25
# speedup vs baseline: 1.0477x; 1.0477x over previous
"""YOLOv1 loss (nn_LossModul_16277926052544) on 8 TRN2 NeuronCores.

Pure data parallel: batch 8192 -> 8 shards of 1024. Each core computes a
partial loss over its shard; host sums the 8x128 partials.

v10 design (75us baseline -> 62.7 -> 58.3 -> 53.2 -> this). Findings:
  * ~350ns fixed cost per Vector op (SBUF init + semaphore instr) and a
    ~16us fixed NEFF startup+teardown dominate; one 392-cell tile (T=1)
    minimizes op count.
  * inputs host-cast bf16 (TT 2x_1p, TS 4x; STT/copy_predicated are 1x on
    cayman, so plain TT/TS preferred).  cls channels (20 of 35) are fp8
    e4m3: the cls stream halves again, and quantization only biases the
    cls term ~0.2% (tolerance 2e-2).
  * cls diff (pcls-tcls) computed BY THE DMA: host stores negated
    targets; SWDGE accum_op=add DMAs (SDMA CCE) cast fp8->bf16 and add
    onto the loaded pred rows.  CCE mangles transfers over ~2048
    elems/partition -> 5-row accum chunks.
  * DMA ordering: geometry rows stream first (Vector starts ~9.5us); the
    cls stream is gated behind them by a 1-element Vector write that
    creates a WAW edge (program order alone gets rescheduled).
  * xy rows host-prescaled by R=1/S and |.| taken in-place via a bf16
    sign-bit AND (bitcast u16): squares kill the sign, masks fold the
    7*sqrt5 constant back.
  * no GpSimd compute (each GP op ~1.5-2us fixed, and GP ops block DVE
    2-port ops via the shared SBUF port); GP only issues SWDGE DMAs.
  * ACT holds one table set (warm-up Sqrt during the DMA ramp; Square
    lives in the sqrt set) and runs Sqrt(scale=5 folds lambda) plus three
    Square+accum ops that pipeline behind Vector.
"""
import sys

for _p in ("/opt/trn_rl_repo",):
    if _p not in sys.path:
        sys.path.insert(0, _p)

import numpy as np
import ml_dtypes
from contextlib import ExitStack

import concourse.bass as bass  # noqa: F401  (registers engines)
from concourse import bacc, mybir
from concourse import bass_utils
import concourse.tile as tile

N_CORES = 8
BATCH = 8192
S = 7
P = 128
F = (BATCH // N_CORES) * S * S // P           # 392 cells per partition
R = 1.0 / S
EPS5 = 5e-6                                   # 5 * EPS (lambda folded)
SQRT5_7 = float(7.0 * np.sqrt(5.0))           # xy mask: host prescaled by R
SQH = float(np.sqrt(0.5))

CG = 15                                       # geometry rows per cell
CC = 20                                       # cls rows per cell

f32 = mybir.dt.float32
bf16 = mybir.dt.bfloat16
u16 = mybir.dt.uint16
u32 = mybir.dt.uint32
f8 = mybir.dt.float8e4
Alu = mybir.AluOpType
Act = mybir.ActivationFunctionType

_CACHE = {}


def _build_body(tc, ctx, xg, xc, tn, out_ap):
    nc = tc.nc
    wk = ctx.enter_context(tc.tile_pool(name="wk", bufs=1))
    stats = wk.tile([P, 4], f32)
    eps5c = wk.tile([P, 1], f32)                # bias const for Sqrt
    nc.gpsimd.memset(eps5c[:], EPS5)
    warm = wk.tile([P, 1], f32)                 # pulls the sqrt table set in
    nc.scalar.activation(warm[:], eps5c[:], Act.Sqrt)

    # xg rows: 0:2 R*pxy1 | 2:4 R*pxy2 | 4:7 pw1,ph1,pc1 | 7:10 pw2,ph2,pc2
    #          10:12 tw,th | 12 tc | 13:15 -R*tx,-R*ty
    xp = wk.tile([P, CG, F], bf16, tag="x")
    nc.sync.dma_start(xp[:, 4:15], xg[:, 4:15])
    nc.sync.dma_start(xp[:, 0:4], xg[:, 0:4])

    dcls = wk.tile([P, CC, F], bf16, tag="dcls")
    # Gate the cls stream on the geometry DMAs so it doesn't steal SDMA
    # ring bandwidth from them: a 1-element Vector write into dcls creates
    # the WAW edge the scheduler respects.
    nc.vector.tensor_copy(dcls[:, 0, 0:1], xp[:, 0, 0:1])
    # fp8 pcls cast to bf16 by the load; fp8 -tcls cast+added by the CCE
    # accumulates (5-row chunks stay under the ~2048 elem/partition limit)
    nc.gpsimd.dma_start(dcls[:], xc[:, 0:20])
    for k in range(4):
        nc.gpsimd.dma_start(dcls[:, 5 * k:5 * k + 5],
                            tn[:, 5 * k:5 * k + 5], accum_op=Alu.add)

    pxy = xp[:, 0:4].rearrange("p (b c) f -> p b c f", b=2)
    pbox = xp[:, 4:10].rearrange("p (b c) f -> p b c f", b=2)
    pwh = pbox[:, :, 0:2, :]                                  # [P,2,2,F]
    pc = pbox[:, :, 2, :]                                     # [P,2,F]
    twh = xp[:, 10:12]                                        # [P,2,F]
    twh_b = xp[:, 10:12].unsqueeze(1).broadcast_to([P, 2, 2, F])
    tcf = xp[:, 12]                                           # [P,F]
    ntxy_b = xp[:, 13:15].unsqueeze(1).broadcast_to([P, 2, 2, F])

    def bc(ap_pf, k):
        return ap_pf.unsqueeze(1).broadcast_to([P, k, F])

    flat = lambda a: a.rearrange("p b c f -> p (b c f)")

    # ---------------- masks (TS, 4x) ----------------
    mo = wk.tile([P, F], bf16, tag="mo")
    nc.vector.tensor_scalar(mo[:], tcf, 0.0, None, op0=Alu.is_gt)
    ms5 = wk.tile([P, F], bf16, tag="ms5")
    nc.vector.tensor_scalar(ms5[:], tcf, 0.0, SQRT5_7, op0=Alu.is_gt,
                            op1=Alu.mult)
    mnh = wk.tile([P, F], bf16, tag="mnh")
    nc.vector.tensor_scalar(mnh[:], tcf, 0.0, SQH, op0=Alu.is_le,
                            op1=Alu.mult)

    # ---------------- geometry ----------------
    # dxy = |R*(pxy - txy)|: xy rows are host-prescaled by R, and the xy
    # loss squares the selected value, so the abs can run in place
    dxy = wk.tile([P, 2, 2, F], bf16, tag="dxy")
    nc.vector.tensor_tensor(dxy[:], pxy, ntxy_b, op=Alu.add)
    nc.vector.tensor_scalar(
        flat(dxy[:]).bitcast(u16), flat(dxy[:]).bitcast(u16), 0x7FFF,
        None, op0=Alu.bitwise_and)
    sth = wk.tile([P, 2, F], bf16, tag="sth")          # twh/2
    nc.vector.tensor_scalar(sth[:], twh, 0.5, None, op0=Alu.mult)
    hp = wk.tile([P, 2, 2, F], bf16, tag="hp")         # pwh/2
    nc.vector.tensor_scalar(hp[:], pwh, 0.5, None, op0=Alu.mult)
    s = wk.tile([P, 2, 2, F], bf16, tag="s")           # (pwh+twh)/2
    nc.vector.tensor_tensor(
        s[:], hp[:], sth[:].unsqueeze(1).broadcast_to([P, 2, 2, F]),
        op=Alu.add)
    m = wk.tile([P, 2, 2, F], bf16, tag="m")           # s - R|dxy|
    nc.vector.tensor_sub(flat(m[:]), flat(s[:]), flat(dxy[:]))
    mwh = wk.tile([P, 2, 2, F], bf16, tag="mwh")       # min(pwh, twh)
    nc.vector.tensor_tensor(mwh[:], pwh, twh_b, op=Alu.min)
    ln = wk.tile([P, 2, 2, F], bf16, tag="ln")         # clamped overlap
    nc.vector.scalar_tensor_tensor(flat(ln[:]), flat(m[:]), 0.0,
                                   flat(mwh[:]), op0=Alu.max, op1=Alu.min)

    ID = wk.tile([P, 4, F], bf16, tag="ID")            # I1 I2 D1 D2
    nc.vector.tensor_mul(ID[:, 0:2], ln[:, :, 0, :], ln[:, :, 1, :])
    pA = wk.tile([P, 2, F], bf16, tag="pA")
    nc.vector.tensor_mul(pA[:], pbox[:, :, 0, :], pbox[:, :, 1, :])
    tA = wk.tile([P, 1, F], bf16, tag="tA")
    nc.vector.tensor_mul(tA[:], xp[:, 10:11], xp[:, 11:12])
    PT = wk.tile([P, 2, F], bf16, tag="PT")
    nc.vector.tensor_tensor(PT[:], pA[:], tA[:].broadcast_to([P, 2, F]),
                            op=Alu.add)
    nc.vector.tensor_sub(ID[:, 2:4], PT[:], ID[:, 0:2])

    g = wk.tile([P, 2, F], bf16, tag="g")
    nc.vector.tensor_mul(g[:, 0], ID[:, 0], ID[:, 3])
    nc.vector.tensor_mul(g[:, 1], ID[:, 1], ID[:, 2])
    resp = wk.tile([P, F], u32, tag="resp")            # 1 -> box1
    nc.vector.tensor_tensor(resp[:], g[:, 0], g[:, 1], op=Alu.is_gt)

    # ---------------- select responsible box ----------------
    # the wh select goes first: it feeds the ACT Sqrt round-trip, the
    # longest cross-engine chain hanging off resp
    sel = wk.tile([P, 7, F], bf16, tag="sel")
    nc.vector.tensor_copy(sel[:, 2:5], xp[:, 7:10])    # box2 defaults
    nc.vector.copy_predicated(sel[:, 2:5], bc(resp[:], 3), xp[:, 4:7])
    # work tile rows: 0:20 mo*dcls | 20:22 7sqrt5*mo*dxy_sel | 22:24 dwh
    #                 24 mo*(c_sel-iou) | 25:27 sqrt(.5)*(1-mo)*pc
    W = wk.tile([P, 27, F], bf16, tag="W")
    sq = wk.tile([P, 4, F], bf16, tag="sq")            # mo*selwh | mo*twh
    nc.vector.tensor_mul(sq[:, 0:2], sel[:, 2:4], bc(mo[:], 2))
    nc.vector.tensor_mul(sq[:, 2:4], twh, bc(mo[:], 2))
    # sqrt(5*mo*wh + 5eps): lambda_coord folds into the free affine
    nc.scalar.activation(sq[:], sq[:], Act.Sqrt, bias=eps5c[:], scale=5.0)

    nc.vector.tensor_copy(sel[:, 0:2], dxy[:, 1])
    nc.vector.tensor_copy(sel[:, 5:7], ID[:, 1:4:2])
    nc.vector.copy_predicated(sel[:, 0:2], bc(resp[:], 2), dxy[:, 0])
    nc.vector.copy_predicated(sel[:, 5:7], bc(resp[:], 2), ID[:, 0:3:2])

    # first cls half: the DMA-computed diffs streamed in during geometry,
    # and ACT can start its big Square while Vector runs the conf chain
    nc.vector.tensor_mul(W[:, 0:10], dcls[:, 0:10], bc(mo[:], 10))
    nc.scalar.activation(W[:, 0:10], W[:, 0:10], Act.Square,
                         accum_out=stats[:, 1:2])

    nc.vector.tensor_mul(W[:, 20:22], sel[:, 0:2], bc(ms5[:], 2))
    nc.vector.tensor_sub(W[:, 22:24], sq[:, 0:2], sq[:, 2:4])
    Dsel = wk.tile([P, F], f32, tag="Dsel")
    nc.vector.tensor_copy(Dsel[:], sel[:, 6])
    rcp = wk.tile([P, F], f32, tag="rcp")
    nc.vector.reciprocal_approx_fast(rcp[:], Dsel[:])
    iou = wk.tile([P, F], bf16, tag="iou")
    nc.vector.tensor_mul(iou[:], sel[:, 5], rcp[:])
    cd = wk.tile([P, F], bf16, tag="cd")               # c_sel - iou
    nc.vector.tensor_sub(cd[:], sel[:, 4], iou[:])
    nc.vector.tensor_mul(W[:, 24], cd[:], mo[:])
    nc.vector.tensor_mul(W[:, 25:27], pc, bc(mnh[:], 2))
    nc.scalar.activation(W[:, 20:27], W[:, 20:27], Act.Square,
                         accum_out=stats[:, 0:1])

    nc.vector.tensor_mul(W[:, 10:20], dcls[:, 10:20], bc(mo[:], 10))
    nc.scalar.activation(W[:, 10:20], W[:, 10:20], Act.Square,
                         accum_out=stats[:, 2:3])

    nc.vector.tensor_copy(stats[:, 3:4], warm[:])      # keep warm read live
    total = wk.tile([P, 1], f32)
    nc.vector.tensor_reduce(total[:], stats[:, 0:3],
                            axis=mybir.AxisListType.X, op=Alu.add)
    nc.sync.dma_start(out_ap, total[:])


def _build():
    if "nc" in _CACHE:
        return _CACHE["nc"]
    nc = bacc.Bacc("TRN2", target_bir_lowering=False, debug=False)
    xg = nc.dram_tensor("xg", [P, CG, F], bf16, kind="ExternalInput")
    xc = nc.dram_tensor("xc", [P, CC, F], f8, kind="ExternalInput")
    tn = nc.dram_tensor("tn", [P, CC, F], f8, kind="ExternalInput")
    out = nc.dram_tensor("out", [P, 1], f32, kind="ExternalOutput")
    with tile.TileContext(nc) as tc, ExitStack() as ctx:
        _build_body(tc, ctx, xg.ap(), xc.ap(), tn.ap(), out.ap())
    nc.compile()
    _CACHE["nc"] = nc
    return nc


def _shard(predicts, targets):
    """Full f32 inputs -> per-core (xg bf16, xc fp8, tn fp8) arrays."""
    bpc = BATCH // N_CORES
    xgs, xcs, tns = [], [], []
    for i in range(N_CORES):
        p = np.asarray(predicts[i * bpc:(i + 1) * bpc], dtype=np.float32)
        g = np.asarray(targets[i * bpc:(i + 1) * bpc], dtype=np.float32)
        pm = np.moveaxis(p.reshape(P, F, 30), 2, 1)   # [P,30,F]
        gm = np.moveaxis(g.reshape(P, F, 30), 2, 1)
        xg = np.empty((P, CG, F), dtype=np.float32)
        xg[:, 0:2] = R * pm[:, 0:2]     # R*pxy1
        xg[:, 2:4] = R * pm[:, 5:7]     # R*pxy2
        xg[:, 4:7] = pm[:, 2:5]         # pw1 ph1 pc1
        xg[:, 7:10] = pm[:, 7:10]       # pw2 ph2 pc2
        xg[:, 10:12] = gm[:, 2:4]       # tw th
        xg[:, 12] = gm[:, 4]            # tconf
        xg[:, 13:15] = -R * gm[:, 0:2]  # -R*tx -R*ty
        xgs.append(xg.astype(ml_dtypes.bfloat16))
        xcs.append(np.ascontiguousarray(pm[:, 10:30])
                   .astype(ml_dtypes.float8_e4m3))
        tns.append(np.ascontiguousarray(-gm[:, 10:30])
                   .astype(ml_dtypes.float8_e4m3))
    return xgs, xcs, tns


def run(predicts, targets, trace=False, **trace_kwargs):
    nc = _build()
    xgs, xcs, tns = _shard(predicts, targets)
    in_maps = [{"xg": xgs[i], "xc": xcs[i], "tn": tns[i]}
               for i in range(N_CORES)]
    res = bass_utils.run_bass_kernel_spmd(
        nc, in_maps, core_ids=list(range(N_CORES)), trace=trace,
        **trace_kwargs)
    partial = np.zeros((), dtype=np.float64)
    for r in res.results:
        partial += np.asarray(r["out"], dtype=np.float64).sum()
    return np.float32(partial), res


def kernel(predicts, targets):
    out, _ = run(predicts, targets, trace=False)
    return out
